# revision 3
# baseline (speedup 1.0000x reference)
import numpy as np

NV = 100000
NTOT = 200000
C = 2048
CPC = 256            # clusters per core
NCORES = 8
SEND_REAL = CPC * 128                # 32768 h rows per core
SEND_ROWS = SEND_REAL + 128          # + zero block
GAMMA = 1.0
SCALE = 8.0          # sqrt(64)

_cache = {}


def _build(BPC, CHUNKS_C):
    import concourse.bass as bass
    import concourse.mybir as mybir
    import concourse.tile as tile
    import concourse.bacc as bacc
    from concourse.masks import make_identity

    f32 = mybir.dt.float32
    bf16 = mybir.dt.bfloat16
    fp8 = mybir.dt.float8e4
    i32 = mybir.dt.int32
    u8 = mybir.dt.uint8
    NBLK = CHUNKS_C * BPC
    ROWS = CHUNKS_C * 128            # compact table rows per core
    NPAD_C = NCORES * ROWS

    # packed per-core blob: bf16 table | i32 offs (xg|mrg) | f32 (Baug|Wv|Wo|rc) | u8 ids
    TB = ROWS * 65 * 2
    OB = 128 * (CPC + NBLK) * 4
    WB = (66 * 66 + 64 * 64 + 64 * 64) * 4
    RB = 128 * CHUNKS_C * 4
    IB = 128 * NBLK
    BLOB = TB + OB + WB + RB + IB

    nc = bacc.Bacc("TRN2", target_bir_lowering=False, debug=False)
    blob = nc.dram_tensor("blob", [BLOB], u8, kind="ExternalInput")
    out_sh = nc.dram_tensor("out_sh", [ROWS, 64], fp8, kind="ExternalOutput")

    psend = nc.dram_tensor("psend", [ROWS, 65], bf16)
    ptab = nc.dram_tensor("ptab", [NPAD_C, 65], bf16)
    send = nc.dram_tensor("send", [SEND_ROWS, 64], f32)
    allh = nc.dram_tensor("allh", [NCORES * SEND_ROWS, 64], f32)

    with tile.TileContext(nc) as tc:
        with tc.tile_pool(name="const", bufs=1) as cp:
            ident = cp.tile([128, 128], f32)
            make_identity(nc, ident[:])
            iot_i = cp.tile([128, 128], i32)
            nc.gpsimd.iota(out=iot_i[:], pattern=[[1, 128]], base=0, channel_multiplier=0)
            iot_f = cp.tile([128, 128], f32)
            nc.vector.tensor_copy(out=iot_f[:], in_=iot_i[:])
            wts = blob[TB + OB:TB + OB + WB].bitcast(f32)
            ba_sb = cp.tile([66, 66], f32)
            nc.sync.dma_start(out=ba_sb[:], in_=wts[0:66 * 66].rearrange("(p c) -> p c", p=66))
            wv_sb = cp.tile([64, 64], f32)
            nc.sync.dma_start(out=wv_sb[:],
                              in_=wts[66 * 66:66 * 66 + 64 * 64].rearrange("(p c) -> p c", p=64))
            wo_sb = cp.tile([64, 64], f32)
            nc.sync.dma_start(out=wo_sb[:],
                              in_=wts[66 * 66 + 64 * 64:66 * 66 + 2 * 64 * 64]
                              .rearrange("(p c) -> p c", p=64))
            rc_sb = cp.tile([128, CHUNKS_C], f32)
            nc.sync.dma_start(out=rc_sb[:],
                              in_=blob[TB + OB + WB:TB + OB + WB + RB].bitcast(f32)
                              .rearrange("(p c) -> p c", p=128))
            offs = blob[TB:TB + OB].bitcast(i32).rearrange("(p c) -> p c", p=128)
            xo_sb = cp.tile([128, CPC], i32)
            nc.sync.dma_start(out=xo_sb[:], in_=offs[:, 0:CPC])
            mo_sb = cp.tile([128, NBLK], i32)
            nc.sync.dma_start(out=mo_sb[:], in_=offs[:, CPC:CPC + NBLK])
            id_u8sb = cp.tile([128, NBLK], u8)
            nc.sync.dma_start(out=id_u8sb[:],
                              in_=blob[TB + OB + WB + RB:BLOB].rearrange("(p c) -> p c", p=128))
            id_sb = cp.tile([128, NBLK], f32)
            nc.vector.tensor_copy(out=id_sb[:], in_=id_u8sb[:])

            # ---------- exchange 0: reassemble compact node table on device ----------
            nc.sync.dma_start(out=psend[:],
                              in_=blob[0:TB].bitcast(bf16).rearrange("(r c) -> r c", r=ROWS))
            nc.gpsimd.collective_compute(
                "AllGather", mybir.AluOpType.bypass,
                replica_groups=[list(range(NCORES))],
                ins=[psend[:]], outs=[ptab[:]])

            # ---------- phase A: per-cluster attention, 4 clusters per group ----------
            with tc.tile_pool(name="asb", bufs=3) as asb, \
                 tc.tile_pool(name="aps", bufs=1, space="PSUM") as aps, \
                 tc.tile_pool(name="aps2", bufs=2, space="PSUM") as aps2, \
                 tc.tile_pool(name="xt4p", bufs=2) as xt4p, \
                 tc.tile_pool(name="xgp", bufs=3) as xgp:
                for g in range(CPC // 4):
                    xgb4 = xgp.tile([128, 4, 65], bf16, tag="xgb")
                    for c4 in range(4):
                        c = g * 4 + c4
                        nc.gpsimd.indirect_dma_start(
                            out=xgb4[:, c4, :], out_offset=None, in_=ptab[:],
                            in_offset=bass.IndirectOffsetOnAxis(ap=xo_sb[:, c:c + 1], axis=0))
                    xg4 = xgp.tile([128, 4, 66], f32, tag="xg")
                    nc.any.tensor_copy(out=xg4[:, :, 0:65], in_=xgb4[:])
                    nc.gpsimd.memset(xg4[:, :, 65:66], 1.0)
                    tpp = aps.tile([66, 512], f32, tag="tp")
                    for c4 in range(4):
                        nc.tensor.transpose(out=tpp[:, c4 * 128:(c4 + 1) * 128],
                                            in_=xg4[:, c4, :], identity=ident[:])
                    XT4 = xt4p.tile([66, 512], f32)
                    nc.any.tensor_copy(out=XT4[:], in_=tpp[:])
                    P4p = aps.tile([66, 512], f32, tag="p4")
                    nc.tensor.matmul(out=P4p[:], lhsT=ba_sb[:], rhs=XT4[:], start=True, stop=True)
                    P4 = asb.tile([66, 512], f32, tag="p4s")
                    nc.any.tensor_copy(out=P4[:], in_=P4p[:])
                    STb = aps2.tile([128, 512], f32, tag="st")
                    for c4 in range(4):
                        cs = slice(c4 * 128, (c4 + 1) * 128)
                        nc.tensor.matmul(out=STb[:, cs], lhsT=XT4[:, cs], rhs=P4[:, cs],
                                         start=True, stop=True)
                    y2b = asb.tile([128, 512], f32, tag="y2")
                    nc.vector.tensor_scalar(out=y2b[:], in0=STb[:], scalar1=0.2, scalar2=None,
                                            op0=mybir.AluOpType.mult)
                    Lb = asb.tile([128, 512], f32, tag="lr")
                    nc.vector.tensor_tensor(out=Lb[:], in0=STb[:], in1=y2b[:],
                                            op=mybir.AluOpType.max)
                    Eb = asb.tile([128, 512], f32, tag="ex")
                    nc.scalar.activation(out=Eb[:], in_=Lb[:],
                                         func=mybir.ActivationFunctionType.Exp)
                    Vp4 = aps.tile([128, 256], f32, tag="vp")
                    for c4 in range(4):
                        nc.tensor.matmul(out=Vp4[:, c4 * 64:(c4 + 1) * 64],
                                         lhsT=XT4[0:64, c4 * 128:(c4 + 1) * 128],
                                         rhs=wv_sb[:], start=True, stop=True)
                    Vx = asb.tile([128, 4, 65], f32, tag="vx")
                    nc.vector.tensor_copy(out=Vx[:, :, 0:64],
                                          in_=Vp4[:].rearrange("p (c d) -> p c d", c=4))
                    nc.gpsimd.memset(Vx[:, :, 64:65], 1.0)
                    Hb = aps2.tile([128, 4, 65], f32, tag="hp")
                    for c4 in range(4):
                        nc.tensor.matmul(out=Hb[:, c4, :],
                                         lhsT=Eb[:, c4 * 128:(c4 + 1) * 128],
                                         rhs=Vx[:, c4, :], start=True, stop=True)
                    rec4 = asb.tile([128, 4, 1], f32, tag="rec")
                    nc.vector.reciprocal(out=rec4[:], in_=Hb[:, :, 64:65])
                    h4 = asb.tile([128, 4, 64], f32, tag="h4")
                    for c4 in range(4):
                        nc.vector.tensor_scalar_mul(h4[:, c4, :], Hb[:, c4, 0:64],
                                                    rec4[:, c4, :])
                    nc.sync.dma_start(
                        out=send[g * 512:(g + 1) * 512, :].rearrange("(c p) d -> p c d", p=128),
                        in_=h4[:, :, :])
                zz = asb.tile([128, 64], f32, tag="zz")
                nc.gpsimd.memset(zz[:], 0.0)
                nc.sync.dma_start(out=send[SEND_REAL:SEND_ROWS, :], in_=zz[:])

            # ---------- exchange 1 ----------
            nc.gpsimd.collective_compute(
                "AllGather", mybir.AluOpType.bypass,
                replica_groups=[list(range(NCORES))],
                ins=[send[:]], outs=[allh[:]])

            # ---------- phase B: segment-mean + project (residual added on host) ----------
            with tc.tile_pool(name="bsb", bufs=4) as bsb, \
                 tc.tile_pool(name="bps", bufs=2, space="PSUM") as bps:
                for j in range(CHUNKS_C):
                    stgs = []
                    ohs = []
                    for w in range(BPC):
                        b = j * BPC + w
                        stg = bsb.tile([128, 64], f32, tag="stg")
                        nc.gpsimd.indirect_dma_start(
                            out=stg[:, :], out_offset=None, in_=allh[:],
                            in_offset=bass.IndirectOffsetOnAxis(ap=mo_sb[:, b:b + 1], axis=0))
                        stgs.append(stg)
                        oh = bsb.tile([128, 128], f32, tag="oh")
                        nc.vector.tensor_tensor(out=oh[:], in0=id_sb[:, b:b + 1].to_broadcast([128, 128]),
                                                in1=iot_f[:], op=mybir.AluOpType.is_equal)
                        ohs.append(oh)
                    oT = bps.tile([64, 128], f32, tag="ot")
                    for w in range(BPC):
                        nc.tensor.matmul(out=oT[:], lhsT=stgs[w][:, :], rhs=ohs[w][:],
                                         start=(w == 0), stop=(w == BPC - 1))
                    oTs = bsb.tile([64, 128], f32, tag="ots")
                    nc.any.tensor_copy(out=oTs[:], in_=oT[:])
                    fp = bps.tile([128, 64], f32, tag="fp")
                    nc.tensor.matmul(out=fp[:], lhsT=oTs[:], rhs=wo_sb[:], start=True, stop=True)
                    fs = bsb.tile([128, 64], fp8, tag="fs")
                    nc.vector.tensor_scalar_mul(fs[:], fp[:], rc_sb[:, j:j + 1])
                    nc.sync.dma_start(out=out_sh[j * 128:(j + 1) * 128, :], in_=fs[:])

    nc.compile()
    return nc


def _prep(inputs):
    import ml_dtypes
    x_var = np.asarray(inputs["x_var"], np.float32)
    x_clause = np.asarray(inputs["x_clause"], np.float32)
    cvi = np.asarray(inputs["cluster_var_ids"]).astype(np.int64)
    cci = np.asarray(inputs["cluster_clause_ids"]).astype(np.int64)
    sat = np.asarray(inputs["satisfaction_scores"], np.float32)
    W_Q = np.asarray(inputs["W_Q"], np.float32)
    W_K = np.asarray(inputs["W_K"], np.float32)
    W_V = np.asarray(inputs["W_V"], np.float32)
    hww = np.asarray(inputs["head_weights"], np.float32)
    ah = int(inputs["active_heads"])
    Wo = np.asarray(inputs["out_proj_w"], np.float32)
    hw = float(np.mean(hww[:ah]))

    nodes = np.concatenate([cvi, cci + NV], 1)                    # [2048, 128] int64

    B_Tm = (W_Q.T @ W_K / SCALE).astype(np.float32)
    Baug = np.zeros((66, 66), np.float32)
    Baug[0:64, 0:64] = B_Tm
    Baug[65, 64] = 1.0
    W_VTm = (W_V * hw).T.copy().astype(np.float32)
    W_oTm = Wo.T.copy().astype(np.float32)

    flat = nodes.reshape(-1)
    cidx = np.arange(C * 128) // 128
    slot = np.arange(C * 128) % 128
    allh_row = ((cidx // CPC) * SEND_ROWS + (cidx % CPC) * 128 + slot).astype(np.int64)
    order = np.argsort(flat, kind="stable")
    sids = flat[order]
    srows = allh_row[order].astype(np.int32)
    ZROW = SEND_REAL   # core 0's zero block in allh

    # unique referenced ids + counts
    m = np.empty(len(sids), bool)
    m[0] = True
    np.not_equal(sids[1:], sids[:-1], out=m[1:])
    uniq = sids[m]                                                # sorted unique node ids
    NREF = len(uniq)
    first = np.flatnonzero(m)
    counts = np.diff(np.append(first, len(sids)))
    NPAD_C = -(-NREF // (NCORES * 128)) * (NCORES * 128)
    CHUNKS_C = NPAD_C // (NCORES * 128)
    ROWS = CHUNKS_C * 128
    nbkt = NPAD_C // 128

    # balance buckets: snake-distribute ids by descending count so each
    # 128-id bucket's total contribution count stays near the mean
    desc = np.argsort(-counts, kind="stable")                     # uniq indices
    padded = np.full(128 * nbkt, -1, np.int64)
    padded[:NREF] = desc
    stripes = padded.reshape(128, nbkt)
    stripes[1::2] = stripes[1::2, ::-1]
    spos = (np.arange(nbkt)[None, :] * 128 + np.arange(128)[:, None]).ravel()
    sidx = stripes.ravel()
    valid = sidx >= 0
    newpos = np.empty(NREF, np.int64)
    newpos[sidx[valid]] = spos[valid]

    inv = np.zeros(NTOT, np.int32)
    inv[uniq] = newpos.astype(np.int32)
    cpos = inv[sids]
    nvar = int(np.searchsorted(uniq, NV))

    # compact table (rows permuted by newpos)
    ctab = np.zeros((NPAD_C, 65), ml_dtypes.bfloat16)
    ctab[newpos[:nvar], 0:64] = x_var[uniq[:nvar]]
    ctab[newpos[nvar:], 0:64] = x_clause[uniq[nvar:] - NV]
    ctab[newpos[nvar:], 64] = GAMMA * sat[uniq[nvar:] - NV]

    # reciprocal counts per compact row
    rc_by_cid = np.ones(NPAD_C, np.float32)
    rc_by_cid[newpos] = 1.0 / np.maximum(counts, 1).astype(np.float32)

    # bucket layout for phase B
    ord2 = np.argsort(cpos, kind="stable")
    cpos2 = cpos[ord2]
    srows2 = srows[ord2]
    lo = np.searchsorted(cpos2, np.arange(0, NPAD_C, 128))
    hi = np.searchsorted(cpos2, np.arange(128, NPAD_C + 128, 128))
    maxc = int(np.max(hi - lo))
    BPC = -(-maxc // 128)
    assert BPC <= 4, maxc
    S = BPC * 128
    NBLK = CHUNKS_C * BPC

    bkt = cpos2 >> 7
    rank = np.arange(len(cpos2)) - lo[bkt]
    pos = bkt * S + rank
    mrg_g = np.full((NCORES * CHUNKS_C * S,), ZROW, np.int32)
    ids_g = np.full((NCORES * CHUNKS_C * S,), 255, np.uint8)
    mrg_g[pos] = srows2
    ids_g[pos] = (cpos2 & 127).astype(np.uint8)

    xg_all = inv[nodes].astype(np.int32)                          # [2048, 128] compact ids

    TB = ROWS * 65 * 2
    OB = 128 * (CPC + NBLK) * 4
    WB = (66 * 66 + 64 * 64 + 64 * 64) * 4
    RB = 128 * CHUNKS_C * 4
    IB = 128 * NBLK
    BLOB = TB + OB + WB + RB + IB
    wts_bytes = np.concatenate([Baug.reshape(-1), W_VTm.reshape(-1),
                                W_oTm.reshape(-1)]).view(np.uint8)

    in_maps = []
    for i in range(NCORES):
        xg_o = np.ascontiguousarray(xg_all[i * CPC:(i + 1) * CPC].T)
        mrg_pm = np.ascontiguousarray(
            mrg_g[i * CHUNKS_C * S:(i + 1) * CHUNKS_C * S].reshape(NBLK, 128).T)
        ids_pm = np.ascontiguousarray(
            ids_g[i * CHUNKS_C * S:(i + 1) * CHUNKS_C * S].reshape(NBLK, 128).T)
        rc_pm = np.ascontiguousarray(
            rc_by_cid[i * ROWS:(i + 1) * ROWS].reshape(CHUNKS_C, 128).T)
        bl = np.empty(BLOB, np.uint8)
        bl[0:TB] = ctab[i * ROWS:(i + 1) * ROWS].view(np.uint8).reshape(-1)
        bl[TB:TB + OB] = np.hstack([xg_o, mrg_pm]).view(np.uint8).reshape(-1)
        bl[TB + OB:TB + OB + WB] = wts_bytes
        bl[TB + OB + WB:TB + OB + WB + RB] = rc_pm.view(np.uint8).reshape(-1)
        bl[TB + OB + WB + RB:BLOB] = ids_pm.reshape(-1)
        in_maps.append(dict(blob=bl))
    return in_maps, BPC, CHUNKS_C, uniq, newpos, nvar, NREF


def run(inputs, want_results=False):
    from concourse.bass_utils import run_bass_kernel_spmd
    in_maps, BPC, CHUNKS_C, uniq, newpos, nvar, NREF = _prep(inputs)
    key = (BPC, CHUNKS_C)
    if key not in _cache:
        _cache[key] = _build(BPC, CHUNKS_C)
    nc = _cache[key]
    res = run_bass_kernel_spmd(nc, in_maps, core_ids=list(range(NCORES)))
    upd = np.concatenate([np.asarray(res.results[i]["out_sh"]) for i in range(NCORES)],
                         0).astype(np.float32)
    bo = np.asarray(inputs["out_proj_b"], np.float32)
    out_var = np.asarray(inputs["x_var"], np.float32) + bo[None, :]
    out_cl = np.asarray(inputs["x_clause"], np.float32) + bo[None, :]
    out_var[uniq[:nvar]] += upd[newpos[:nvar]]
    out_cl[uniq[nvar:] - NV] += upd[newpos[nvar:]]
    out = (out_var, out_cl)
    if want_results:
        return out, res
    return out


def kernel(**inputs):
    return run(inputs)
